# revision 2
# baseline (speedup 1.0000x reference)
"""Trainium2 Bass kernel: EpisodicLearningSystem retrieval_knn (8-core SPMD).

Self-contained: kernel(**inputs) takes full unsharded inputs, distributes
across 8 NeuronCores, returns (retrieved_values, confidences, top_sims).
"""

import sys

for _p in ("/opt/trn_rl_repo", "/root/.axon_site/_ro/trn_rl_repo"):
    if _p not in sys.path:
        sys.path.insert(0, _p)

import numpy as np

import concourse.bacc as bacc
import concourse.bass as bass
import concourse.mybir as mybir
from concourse.ordered_set import OrderedSet
from concourse.tile import TileContext
from concourse.masks import make_identity
from concourse.bass_utils import run_bass_kernel_spmd

F32 = mybir.dt.float32
I32 = mybir.dt.int32
I16 = mybir.dt.int16
U32 = mybir.dt.uint32
SP = mybir.EngineType.SP
EPS = 1e-8

# Problem sizes (hardcoded per spec)
B, K, V, N, M, TOPK, CORES = 256, 256, 256, 4000, 4000, 8, 8
BL = B // CORES          # 32 queries per core
NL = N // CORES          # 500 slots per core (extraction shard)
PAIRS = BL * TOPK        # 256 retrieval pairs per core


def _build(xsplit=2):
    nc = bacc.Bacc(enable_partition_id=True, num_devices=CORES)

    qT = nc.dram_tensor("qT", [K, BL], F32, kind="ExternalInput")
    W0 = nc.dram_tensor("W0", [K, 128], F32, kind="ExternalInput")
    W1 = nc.dram_tensor("W1", [K, 128], F32, kind="ExternalInput")
    Wt = nc.dram_tensor("Wt", [K, K], F32, kind="ExternalInput")
    bk0 = nc.dram_tensor("bk0", [128, 1], F32, kind="ExternalInput")
    bk1 = nc.dram_tensor("bk1", [128, 1], F32, kind="ExternalInput")
    bkrow = nc.dram_tensor("bkrow", [1, K], F32, kind="ExternalInput")
    A = nc.dram_tensor("A", [N, K, V], F32, kind="ExternalInput")
    mi32 = nc.dram_tensor("mi32", [M, 1], I32, kind="ExternalInput")
    mi16w = nc.dram_tensor("mi16w", [BL, M // 16], I16, kind="ExternalInput")
    usage = nc.dram_tensor("usage", [N, 1], F32, kind="ExternalInput")

    rv_out = nc.dram_tensor("rv", [PAIRS, V], F32, kind="ExternalOutput")
    conf_out = nc.dram_tensor("conf", [BL, TOPK], F32, kind="ExternalOutput")
    tsim_out = nc.dram_tensor("tsim", [BL, TOPK], F32, kind="ExternalOutput")

    Aflat = A.ap().rearrange("n k v -> (n k v)")

    with TileContext(nc) as tc:
        with (
            tc.tile_pool(name="con", bufs=1) as con,
            tc.tile_pool(name="big", bufs=1) as bigp,
            tc.tile_pool(name="work", bufs=3) as work,
            tc.tile_pool(name="slotp", bufs=8) as slotp,
            tc.tile_pool(name="psum", bufs=2, space="PSUM") as psum,
            tc.tile_pool(name="psumr", bufs=1, space="PSUM") as psumr,
            tc.tile_pool(name="dram", bufs=1, space="DRAM") as dram,
        ):
            pid = nc.partition_id(
                engines=OrderedSet([SP, mybir.EngineType.Activation]))

            # ---- extraction of local keycolT shard -> DRAM ----
            kc_loc = dram.tile([2, 128, NL], F32)
            ag_out = dram.tile([2 * CORES, 128, NL], F32)
            base = pid * (NL * K * V)
            src4 = Aflat[bass.ds(base, NL * K * V)].rearrange(
                "(n p c v) -> c p n v", n=NL, p=128, c=2, v=V)
            nsub = NL // xsplit
            for c in range(2):
                for s in range(xsplit):
                    eng = nc.sync if (c * xsplit + s) % 2 == 0 else nc.scalar
                    eng.dma_start(
                        out=kc_loc[c, :, s * nsub:(s + 1) * nsub],
                        in_=src4[c, :, s * nsub:(s + 1) * nsub, 0])

            # ---- AllGather keycolT shards ----
            nc.gpsimd.collective_compute(
                "AllGather", mybir.AluOpType.bypass,
                replica_groups=[list(range(CORES))],
                ins=[kc_loc.opt()], outs=[ag_out.opt()])

            knT = [bigp.tile([128, N], F32, tag=f"knT{c}", name=f"knT{c}")
                   for c in range(2)]
            for cb in range(CORES):
                for c in range(2):
                    nc.sync.dma_start(out=knT[c][:, cb * NL:(cb + 1) * NL],
                                      in_=ag_out[2 * cb + c])

            # ---- constants ----
            ones_col = con.tile([128, 1], F32)
            nc.vector.memset(ones_col[:], 1.0)
            ones_row = con.tile([1, 128], F32)
            nc.vector.memset(ones_row[:], 1.0)
            ones_bl = con.tile([1, BL], F32)
            nc.vector.memset(ones_bl[:], 1.0)
            ident = con.tile([128, 128], F32)
            make_identity(nc, ident)

            # ---- key norms -> scale knT in place ----
            BLKN = 500
            nblk = N // BLKN
            nrm_row = con.tile([1, N], F32)
            for blk in range(nblk):
                sl = slice(blk * BLKN, (blk + 1) * BLKN)
                pn = psum.tile([1, BLKN], F32, tag="psmall", name="pn")
                for c in range(2):
                    sq = work.tile([128, BLKN], F32, tag="sq")
                    nc.scalar.square(sq[:], knT[c][:, sl])
                    nc.tensor.matmul(out=pn[:], lhsT=ones_col[:], rhs=sq[:],
                                     start=(c == 0), stop=(c == 1))
                nc.vector.tensor_copy(out=nrm_row[:, sl], in_=pn[:])
            nc.scalar.sqrt(nrm_row[:], nrm_row[:])
            nc.vector.tensor_scalar_max(nrm_row[:], nrm_row[:], EPS)
            nc.vector.reciprocal(nrm_row[:], nrm_row[:])
            for blk in range(nblk):
                sl = slice(blk * BLKN, (blk + 1) * BLKN)
                pb = psum.tile([128, BLKN], F32, tag="pblk", name="pb")
                nc.tensor.matmul(out=pb[:], lhsT=ones_row[:], rhs=nrm_row[:, sl],
                                 start=True, stop=True)
                for c in range(2):
                    nc.vector.tensor_mul(out=knT[c][:, sl], in0=knT[c][:, sl],
                                         in1=pb[:])

            # ---- encoder ----
            qTt = con.tile([128, 2, BL], F32)
            nc.sync.dma_start(out=qTt[:], in_=qT.ap().rearrange("(c p) b -> p c b", p=128))
            w0t = con.tile([128, 2, 128], F32)
            nc.sync.dma_start(out=w0t[:], in_=W0.ap().rearrange("(c p) k -> p c k", p=128))
            w1t = con.tile([128, 2, 128], F32)
            nc.sync.dma_start(out=w1t[:], in_=W1.ap().rearrange("(c p) k -> p c k", p=128))
            wtt = con.tile([128, 2, K], F32)
            nc.sync.dma_start(out=wtt[:], in_=Wt.ap().rearrange("(c p) k -> p c k", p=128))
            bk0t = con.tile([128, 1], F32)
            nc.sync.dma_start(out=bk0t[:], in_=bk0[:])
            bk1t = con.tile([128, 1], F32)
            nc.sync.dma_start(out=bk1t[:], in_=bk1[:])
            bkrt = con.tile([1, K], F32)
            nc.sync.dma_start(out=bkrt[:], in_=bkrow[:])

            eT = [con.tile([128, BL], F32, tag=f"eT{c}", name=f"eT{c}")
                  for c in range(2)]
            for c, wt_ in ((0, w0t), (1, w1t)):
                pe = psum.tile([128, BL], F32, tag="psmall", name="pe")
                for jc in range(2):
                    nc.tensor.matmul(out=pe[:], lhsT=wt_[:, jc, :],
                                     rhs=qTt[:, jc, :],
                                     start=(jc == 0), stop=(jc == 1))
                nc.scalar.activation(eT[c][:], pe[:],
                                     mybir.ActivationFunctionType.Tanh,
                                     bias=(bk0t[:, 0:1] if c == 0 else bk1t[:, 0:1]))

            peb = psum.tile([BL, K], F32, tag="psmall", name="peb")
            for jc in range(2):
                nc.tensor.matmul(out=peb[:], lhsT=qTt[:, jc, :], rhs=wtt[:, jc, :],
                                 start=(jc == 0), stop=False)
            nc.tensor.matmul(out=peb[:], lhsT=ones_bl[:], rhs=bkrt[:],
                             start=False, stop=True)
            e_b = con.tile([BL, K], F32)
            nc.scalar.activation(e_b[:], peb[:], mybir.ActivationFunctionType.Tanh)
            esq = work.tile([BL, K], F32, tag="esq")
            nc.scalar.square(esq[:], e_b[:])
            rq = con.tile([BL, 1], F32)
            nc.vector.reduce_sum(out=rq[:], in_=esq[:], axis=mybir.AxisListType.X)
            nc.scalar.sqrt(rq[:], rq[:])
            nc.vector.tensor_scalar_max(rq[:], rq[:], EPS)
            nc.vector.reciprocal(rq[:], rq[:])

            # ---- slot sims ----
            ssims = bigp.tile([BL, N], F32)
            SBLK = 500
            for blk in range(N // SBLK):
                sl = slice(blk * SBLK, (blk + 1) * SBLK)
                ps = psum.tile([BL, SBLK], F32, tag="pblk", name="ps")
                for c in range(2):
                    nc.tensor.matmul(out=ps[:], lhsT=eT[c][:], rhs=knT[c][:, sl],
                                     start=(c == 0), stop=(c == 1))
                nc.vector.tensor_scalar_mul(ssims[:, sl], ps[:], rq[:, 0:1])

            # ---- expand to m-space (memory_index gather) ----
            mi16t = con.tile([BL, M // 16], I16)
            nc.sync.dma_start(out=mi16t[:], in_=mi16w[:])
            msims = bigp.tile([BL, M], F32)
            nc.gpsimd.ap_gather(
                msims[:].rearrange("p (m one) -> p m one", one=1),
                ssims[:].rearrange("p (n one) -> p n one", one=1),
                mi16t[:], channels=BL, num_elems=N, d=1, num_idxs=M)

            # ---- top-8 ----
            tsim = con.tile([BL, 8], F32)
            msel = con.tile([BL, 8], U32)
            nc.vector.max(out=tsim[:], in_=msims[:])
            nc.vector.max_index(out=msel[:], in_max=tsim[:], in_values=msims[:])
            nc.sync.dma_start(out=tsim_out[:], in_=tsim[:])

            # ---- actual slot ids, usage, confidences ----
            act_t = con.tile([BL, 8, 1], I32)
            for kk in range(8):
                nc.gpsimd.indirect_dma_start(
                    out=act_t[:, kk], out_offset=None, in_=mi32[:],
                    in_offset=bass.IndirectOffsetOnAxis(ap=msel[:, kk:kk + 1], axis=0))
            usel = con.tile([BL, 8, 1], F32)
            for kk in range(8):
                nc.gpsimd.indirect_dma_start(
                    out=usel[:, kk], out_offset=None, in_=usage[:],
                    in_offset=bass.IndirectOffsetOnAxis(ap=act_t[:, kk, 0:1], axis=0))
            lus = con.tile([BL, 8], F32)
            nc.scalar.activation(lus[:], usel[:, :, 0],
                                 mybir.ActivationFunctionType.Ln, bias=1.0)
            nc.vector.tensor_scalar_add(lus[:], lus[:], 1.0)
            conf = con.tile([BL, 8], F32)
            nc.vector.tensor_mul(out=conf[:], in0=tsim[:], in1=lus[:])
            nc.sync.dma_start(out=conf_out[:], in_=conf[:])

            # ---- retrieval: dynamic-offset slot DMAs + PE matvecs ----
            pr = [psumr.tile([128, PAIRS], F32, tag=f"pr{vh}", name=f"pr{vh}")
                  for vh in range(2)]
            for b in range(BL):
                for kk in range(TOPK):
                    j = b * TOPK + kk
                    with nc.sync.register(f"ract{j}") as r:
                        nc.sync.reg_load(r, act_t[b:b + 1, kk, 0:1])
                        sv = nc.snap(r, engines=OrderedSet([SP]),
                                     min_val=0, max_val=N - 1)
                    st = slotp.tile([128, 2, V], F32, tag="slot", name="st")
                    nc.sync.dma_start(
                        out=st[:],
                        in_=Aflat[bass.ds(sv * (K * V), K * V)].rearrange(
                            "(p c v) -> p c v", p=128, c=2, v=V))
                    for vh in range(2):
                        for c in range(2):
                            nc.tensor.matmul(
                                out=pr[vh][:, j:j + 1],
                                lhsT=st[:, c, vh * 128:(vh + 1) * 128],
                                rhs=eT[c][:, b:b + 1],
                                start=(c == 0), stop=(c == 1))

            # ---- transpose + write retrieved values ----
            rT = []
            for vh in range(2):
                t = bigp.tile([128, PAIRS], F32, tag=f"rT{vh}", name=f"rT{vh}")
                nc.vector.tensor_copy(out=t[:], in_=pr[vh][:])
                rT.append(t)
            for pc in range(PAIRS // 128):
                rvs = work.tile([128, V], F32, tag="rvs")
                for vh in range(2):
                    pt = psum.tile([128, 128], F32, tag="psmall", name="pt")
                    nc.tensor.transpose(out=pt[:],
                                        in_=rT[vh][:, pc * 128:(pc + 1) * 128],
                                        identity=ident[:])
                    nc.vector.tensor_copy(out=rvs[:, vh * 128:(vh + 1) * 128],
                                          in_=pt[:])
                nc.sync.dma_start(out=rv_out[pc * 128:(pc + 1) * 128, :], in_=rvs[:])

    nc.finalize()
    return nc


_NC_CACHE = None


def _get_nc():
    global _NC_CACHE
    if _NC_CACHE is None:
        _NC_CACHE = _build()
    return _NC_CACHE


def make_in_maps(query_key, W_k, b_k, assoc_matrix, usage_counter, memory_index):
    q = np.asarray(query_key, dtype=np.float32)
    W = np.asarray(W_k, dtype=np.float32)
    b = np.asarray(b_k, dtype=np.float32)
    A = np.ascontiguousarray(np.asarray(assoc_matrix, dtype=np.float32))
    usage = np.asarray(usage_counter, dtype=np.float32)
    mi = np.asarray(memory_index).astype(np.int32)

    Wt = np.ascontiguousarray(W.T)
    W0 = np.ascontiguousarray(Wt[:, 0::2])
    W1 = np.ascontiguousarray(Wt[:, 1::2])
    bk0 = np.ascontiguousarray(b[0::2][:, None])
    bk1 = np.ascontiguousarray(b[1::2][:, None])
    bkrow = np.ascontiguousarray(b[None, :])
    mi32 = np.ascontiguousarray(mi[:, None])
    w16 = np.ascontiguousarray(mi.astype(np.int16).reshape(M // 16, 16).T)
    mi16w = np.ascontiguousarray(np.tile(w16, (BL // 16, 1)))
    usage2 = np.ascontiguousarray(usage[:, None])

    in_maps = []
    for c in range(CORES):
        qTl = np.ascontiguousarray(q[c * BL:(c + 1) * BL].T)
        in_maps.append({
            "qT": qTl, "W0": W0, "W1": W1, "Wt": Wt,
            "bk0": bk0, "bk1": bk1, "bkrow": bkrow,
            "A": A, "mi32": mi32, "mi16w": mi16w, "usage": usage2,
        })
    return in_maps


def assemble(results):
    rv = np.concatenate([r["rv"].reshape(BL, TOPK, V) for r in results], axis=0)
    conf = np.concatenate([r["conf"] for r in results], axis=0)
    tsim = np.concatenate([r["tsim"] for r in results], axis=0)
    return rv, conf, tsim


def kernel(query_key, W_k, b_k, assoc_matrix, usage_counter, memory_index,
           top_k=8, **_ignored):
    assert int(top_k) == TOPK
    nc = _get_nc()
    in_maps = make_in_maps(query_key, W_k, b_k, assoc_matrix, usage_counter,
                           memory_index)
    res = run_bass_kernel_spmd(nc, in_maps, list(range(CORES)))
    return assemble(res.results)


# revision 3
# speedup vs baseline: 1.0370x; 1.0370x over previous
"""Trainium2 Bass kernel: EpisodicLearningSystem retrieval_knn (8-core SPMD).

Self-contained: kernel(**inputs) takes the full unsharded inputs, shards and
replicates across 8 NeuronCores, runs the Bass/Tile kernel, and returns
(retrieved_values, confidences, top_sims) matching the reference.
"""

import sys

for _p in ("/opt/trn_rl_repo", "/root/.axon_site/_ro/trn_rl_repo"):
    if _p not in sys.path:
        sys.path.insert(0, _p)



import numpy as np

import concourse.bacc as bacc
import concourse.bass as bass
import concourse.mybir as mybir
from concourse.ordered_set import OrderedSet
from concourse.tile import TileContext
from concourse.masks import make_identity

F32 = mybir.dt.float32
I32 = mybir.dt.int32
I16 = mybir.dt.int16
U32 = mybir.dt.uint32
SP = mybir.EngineType.SP
EPS = 1e-8


def build(B, K, V, N, M, TOPK, CORES, xsplit=2, upto='all'):
    assert K == 256 and V == 256 and TOPK == 8
    BL = B // CORES
    NL = N // CORES
    PAIRS = BL * TOPK
    assert BL % 16 == 0 and PAIRS % 128 == 0

    nc = bacc.Bacc(enable_partition_id=True, num_devices=CORES)

    qT = nc.dram_tensor("qT", [K, BL], F32, kind="ExternalInput")
    W0 = nc.dram_tensor("W0", [K, 128], F32, kind="ExternalInput")   # Wt cols k=2p
    W1 = nc.dram_tensor("W1", [K, 128], F32, kind="ExternalInput")   # Wt cols k=2p+1
    Wt = nc.dram_tensor("Wt", [K, K], F32, kind="ExternalInput")
    bk0 = nc.dram_tensor("bk0", [128, 1], F32, kind="ExternalInput")
    bk1 = nc.dram_tensor("bk1", [128, 1], F32, kind="ExternalInput")
    bkrow = nc.dram_tensor("bkrow", [1, K], F32, kind="ExternalInput")
    A = nc.dram_tensor("A", [N, K, V], F32, kind="ExternalInput")
    mi32 = nc.dram_tensor("mi32", [M, 1], I32, kind="ExternalInput")
    mi16w = nc.dram_tensor("mi16w", [BL, M // 16], I16, kind="ExternalInput")
    usage = nc.dram_tensor("usage", [N, 1], F32, kind="ExternalInput")

    rv_out = nc.dram_tensor("rv", [PAIRS, V], F32, kind="ExternalOutput")
    conf_out = nc.dram_tensor("conf", [BL, TOPK], F32, kind="ExternalOutput")
    tsim_out = nc.dram_tensor("tsim", [BL, TOPK], F32, kind="ExternalOutput")

    Aflat = A.ap().rearrange("n k v -> (n k v)")

    with TileContext(nc) as tc:
        with (
            tc.tile_pool(name="con", bufs=1) as con,
            tc.tile_pool(name="big", bufs=1) as bigp,
            tc.tile_pool(name="work", bufs=3) as work,
            tc.tile_pool(name="slotp", bufs=8) as slotp,
            tc.tile_pool(name="psum", bufs=2, space="PSUM") as psum,
            tc.tile_pool(name="psumr", bufs=1, space="PSUM") as psumr,
            tc.tile_pool(name="dram", bufs=1, space="DRAM") as dram,
        ):
            pid = nc.partition_id(
                engines=OrderedSet([SP, mybir.EngineType.Activation]))

            # ---- Phase X: extraction of local keycolT shard -> DRAM ----
            kc_loc = dram.tile([2, 128, NL], F32)
            ag_out = dram.tile([2 * CORES, 128, NL], F32)
            # flat = n*K*V + (2p+c)*V + v ; shard base = pid*NL*K*V
            base = pid * (NL * K * V)
            src4 = Aflat[bass.ds(base, NL * K * V)].rearrange(
                "(n p c v) -> c p n v", n=NL, p=128, c=2, v=V)
            nsub = NL // xsplit
            for c in range(2):
                for s in range(xsplit):
                    eng = nc.sync if (c * xsplit + s) % 2 == 0 else nc.scalar
                    eng.dma_start(
                        out=kc_loc[c, :, s * nsub:(s + 1) * nsub],
                        in_=src4[c, :, s * nsub:(s + 1) * nsub, 0])

            # ---- Phase AG ----
            nc.gpsimd.collective_compute(
                "AllGather", mybir.AluOpType.bypass,
                replica_groups=[list(range(CORES))],
                ins=[kc_loc.opt()], outs=[ag_out.opt()])

            # ---- load full knT ----
            knT = [bigp.tile([128, N], F32, tag=f"knT{c}", name=f"knT{c}") for c in range(2)]
            for cb in range(CORES):
                for c in range(2):
                    nc.sync.dma_start(out=knT[c][:, cb * NL:(cb + 1) * NL],
                                      in_=ag_out[2 * cb + c])

            # ---- constants ----
            ones_col = con.tile([128, 1], F32)
            nc.vector.memset(ones_col[:], 1.0)
            ones_row = con.tile([1, 128], F32)
            nc.vector.memset(ones_row[:], 1.0)
            ones_bl = con.tile([1, BL], F32)
            nc.vector.memset(ones_bl[:], 1.0)
            ident = con.tile([128, 128], F32)
            make_identity(nc, ident)

            # ---- Phase N: key norms -> scale knT in place ----
            BLKN = 500 if N % 500 == 0 else NL
            nblk = N // BLKN
            nrm_row = con.tile([1, N], F32)
            for blk in range(nblk):
                sl = slice(blk * BLKN, (blk + 1) * BLKN)
                pn = psum.tile([1, BLKN], F32, tag="psmall", name="pn")
                for c in range(2):
                    sq = work.tile([128, BLKN], F32, tag="sq")
                    nc.scalar.square(sq[:], knT[c][:, sl])
                    nc.tensor.matmul(out=pn[:], lhsT=ones_col[:], rhs=sq[:],
                                     start=(c == 0), stop=(c == 1))
                nc.vector.tensor_copy(out=nrm_row[:, sl], in_=pn[:])
            nc.scalar.sqrt(nrm_row[:], nrm_row[:])
            nc.vector.tensor_scalar_max(nrm_row[:], nrm_row[:], EPS)
            nc.vector.reciprocal(nrm_row[:], nrm_row[:])
            for blk in range(nblk):
                sl = slice(blk * BLKN, (blk + 1) * BLKN)
                pb = psum.tile([128, BLKN], F32, tag="pblk", name="pb")
                nc.tensor.matmul(out=pb[:], lhsT=ones_row[:], rhs=nrm_row[:, sl],
                                 start=True, stop=True)
                for c in range(2):
                    nc.vector.tensor_mul(out=knT[c][:, sl], in0=knT[c][:, sl], in1=pb[:])

            # ---- Phase E: encoder ----
            qTt = con.tile([128, 2, BL], F32)
            nc.sync.dma_start(out=qTt[:], in_=qT.ap().rearrange("(c p) b -> p c b", p=128))
            w0t = con.tile([128, 2, 128], F32)
            nc.sync.dma_start(out=w0t[:], in_=W0.ap().rearrange("(c p) k -> p c k", p=128))
            w1t = con.tile([128, 2, 128], F32)
            nc.sync.dma_start(out=w1t[:], in_=W1.ap().rearrange("(c p) k -> p c k", p=128))
            wtt = con.tile([128, 2, K], F32)
            nc.sync.dma_start(out=wtt[:], in_=Wt.ap().rearrange("(c p) k -> p c k", p=128))
            bk0t = con.tile([128, 1], F32)
            nc.sync.dma_start(out=bk0t[:], in_=bk0[:])
            bk1t = con.tile([128, 1], F32)
            nc.sync.dma_start(out=bk1t[:], in_=bk1[:])
            bkrt = con.tile([1, K], F32)
            nc.sync.dma_start(out=bkrt[:], in_=bkrow[:])

            eT = [con.tile([128, BL], F32, tag=f"eT{c}", name=f"eT{c}") for c in range(2)]
            for c, wt_ in ((0, w0t), (1, w1t)):
                pe = psum.tile([128, BL], F32, tag="psmall", name="pe")
                for jc in range(2):
                    nc.tensor.matmul(out=pe[:], lhsT=wt_[:, jc, :], rhs=qTt[:, jc, :],
                                     start=(jc == 0), stop=(jc == 1))
                nc.scalar.activation(eT[c][:], pe[:], mybir.ActivationFunctionType.Tanh,
                                     bias=(bk0t[:, 0:1] if c == 0 else bk1t[:, 0:1]))

            # e in [b, k] layout for query norms
            peb = psum.tile([BL, K], F32, tag="psmall", name="peb")
            for jc in range(2):
                nc.tensor.matmul(out=peb[:], lhsT=qTt[:, jc, :], rhs=wtt[:, jc, :],
                                 start=(jc == 0), stop=False)
            nc.tensor.matmul(out=peb[:], lhsT=ones_bl[:], rhs=bkrt[:],
                             start=False, stop=True)
            e_b = con.tile([BL, K], F32)
            nc.scalar.activation(e_b[:], peb[:], mybir.ActivationFunctionType.Tanh)
            esq = work.tile([BL, K], F32, tag="esq")
            nc.scalar.square(esq[:], e_b[:])
            rq = con.tile([BL, 1], F32)
            nc.vector.reduce_sum(out=rq[:], in_=esq[:], axis=mybir.AxisListType.X)
            nc.scalar.sqrt(rq[:], rq[:])
            nc.vector.tensor_scalar_max(rq[:], rq[:], EPS)
            nc.vector.reciprocal(rq[:], rq[:])

            # ---- Phase S: slot sims ----
            ssims = bigp.tile([BL, N], F32)
            SBLK = 500 if N % 500 == 0 else NL
            for blk in range(N // SBLK):
                sl = slice(blk * SBLK, (blk + 1) * SBLK)
                ps = psum.tile([BL, SBLK], F32, tag="pblk", name="ps")
                for c in range(2):
                    nc.tensor.matmul(out=ps[:], lhsT=eT[c][:], rhs=knT[c][:, sl],
                                     start=(c == 0), stop=(c == 1))
                nc.vector.tensor_scalar_mul(ssims[:, sl], ps[:], rq[:, 0:1])

            # ---- Phase G: expand to m-space ----
            mi16t = con.tile([BL, M // 16], I16)
            nc.sync.dma_start(out=mi16t[:], in_=mi16w[:])
            msims = bigp.tile([BL, M], F32)
            nc.gpsimd.ap_gather(
                msims[:].rearrange("p (m one) -> p m one", one=1),
                ssims[:].rearrange("p (n one) -> p n one", one=1),
                mi16t[:], channels=BL, num_elems=N, d=1, num_idxs=M)

            # ---- Phase T: top-8 ----
            tsim = con.tile([BL, 8], F32)
            msel = con.tile([BL, 8], U32)
            nc.vector.max(out=tsim[:], in_=msims[:])
            nc.vector.max_index(out=msel[:], in_max=tsim[:], in_values=msims[:])
            nc.sync.dma_start(out=tsim_out[:], in_=tsim[:])

            # ---- Phase C: actual slots, usage, confidences ----
            act_t = con.tile([BL, 8, 1], I32)
            for kk in range(8):
                nc.gpsimd.indirect_dma_start(
                    out=act_t[:, kk], out_offset=None, in_=mi32[:],
                    in_offset=bass.IndirectOffsetOnAxis(ap=msel[:, kk:kk + 1], axis=0))
            usel = con.tile([BL, 8, 1], F32)
            for kk in range(8):
                nc.gpsimd.indirect_dma_start(
                    out=usel[:, kk], out_offset=None, in_=usage[:],
                    in_offset=bass.IndirectOffsetOnAxis(ap=act_t[:, kk, 0:1], axis=0))
            lus = con.tile([BL, 8], F32)
            nc.scalar.activation(lus[:], usel[:, :, 0],
                                 mybir.ActivationFunctionType.Ln, bias=1.0)
            nc.vector.tensor_scalar_add(lus[:], lus[:], 1.0)
            conf = con.tile([BL, 8], F32)
            nc.vector.tensor_mul(out=conf[:], in0=tsim[:], in1=lus[:])
            nc.sync.dma_start(out=conf_out[:], in_=conf[:])

            # ---- Phase R: retrieval ----
            ACT_E = mybir.EngineType.Activation
            off_t = con.tile([BL, 8, 1], I32)
            if upto == 'all':
                nc.vector.tensor_scalar_mul(off_t[:, :, 0], act_t[:, :, 0], K * V)
            pr = [psumr.tile([128, PAIRS], F32, tag=f"pr{vh}", name=f"pr{vh}")
                  for vh in range(2)] if upto == 'all' else None
            for b in (range(BL) if upto == 'all' else []):
                half = 0 if b < BL // 2 else 1
                eng = nc.sync if half == 0 else nc.scalar
                eng_t = SP if half == 0 else ACT_E
                for kk in range(TOPK):
                    j = b * TOPK + kk
                    with eng.register(f"roff{j}") as r:
                        eng.reg_load(r, off_t[b:b + 1, kk, 0:1])
                        sv = nc.snap(r, engines=OrderedSet([eng_t]),
                                     min_val=0, max_val=(N - 1) * K * V)
                    st = slotp.tile([128, 2, V], F32, tag=f"slot{half}",
                                    name=f"st{half}")
                    eng.dma_start(
                        out=st[:],
                        in_=Aflat[bass.ds(sv, K * V)].rearrange(
                            "(p c v) -> p c v", p=128, c=2, v=V))
                    for vh in range(2):
                        for c in range(2):
                            nc.tensor.matmul(
                                out=pr[vh][:, j:j + 1],
                                lhsT=st[:, c, vh * 128:(vh + 1) * 128],
                                rhs=eT[c][:, b:b + 1],
                                start=(c == 0), stop=(c == 1))

            # ---- Phase O: transpose + write retrieved values ----
            rT = []
            for vh in (range(2) if upto == 'all' else []):
                t = bigp.tile([128, PAIRS], F32, tag=f"rT{vh}", name=f"rT{vh}")
                nc.vector.tensor_copy(out=t[:], in_=pr[vh][:])
                rT.append(t)
            for pc in (range(PAIRS // 128) if upto == 'all' else []):
                rvs = work.tile([128, V], F32, tag="rvs")
                for vh in range(2):
                    pt = psum.tile([128, 128], F32, tag="psmall", name="pt")
                    nc.tensor.transpose(out=pt[:], in_=rT[vh][:, pc * 128:(pc + 1) * 128],
                                        identity=ident[:])
                    nc.vector.tensor_copy(out=rvs[:, vh * 128:(vh + 1) * 128], in_=pt[:])
                nc.sync.dma_start(out=rv_out[pc * 128:(pc + 1) * 128, :], in_=rvs[:])

    nc.finalize()
    return nc


def make_inputs(q, W, b, A, usage, mi, B, K, V, N, M, TOPK, CORES):
    """Build per-core input maps (host-side sharding/layout only)."""
    BL = B // CORES
    q = np.asarray(q, dtype=np.float32)
    W = np.asarray(W, dtype=np.float32)
    b = np.asarray(b, dtype=np.float32)
    A = np.ascontiguousarray(np.asarray(A, dtype=np.float32))
    usage = np.asarray(usage, dtype=np.float32)
    mi = np.asarray(mi).astype(np.int32)

    Wt = np.ascontiguousarray(W.T)
    W0 = np.ascontiguousarray(Wt[:, 0::2])
    W1 = np.ascontiguousarray(Wt[:, 1::2])
    bk0 = np.ascontiguousarray(b[0::2][:, None])
    bk1 = np.ascontiguousarray(b[1::2][:, None])
    bkrow = np.ascontiguousarray(b[None, :])
    mi32 = np.ascontiguousarray(mi[:, None])
    w16 = np.ascontiguousarray(mi.astype(np.int16).reshape(M // 16, 16).T)  # [16, M//16]
    mi16w = np.ascontiguousarray(np.tile(w16, (BL // 16, 1)))
    usage2 = np.ascontiguousarray(usage[:, None])

    in_maps = []
    for c in range(CORES):
        qTl = np.ascontiguousarray(q[c * BL:(c + 1) * BL].T)
        in_maps.append({
            "qT": qTl, "W0": W0, "W1": W1, "Wt": Wt,
            "bk0": bk0, "bk1": bk1, "bkrow": bkrow,
            "A": A, "mi32": mi32, "mi16w": mi16w, "usage": usage2,
        })
    return in_maps


def assemble_outputs(results, B, V, TOPK, CORES):
    BL = B // CORES
    rv = np.concatenate([r["rv"].reshape(BL, TOPK, V) for r in results], axis=0)
    conf = np.concatenate([r["conf"] for r in results], axis=0)
    tsim = np.concatenate([r["tsim"] for r in results], axis=0)
    return rv, conf, tsim


from concourse.bass_utils import run_bass_kernel_spmd

B, K, V, N, M, TOPK, CORES = 256, 256, 256, 4000, 4000, 8, 8

_NC_CACHE = None


def _get_nc():
    global _NC_CACHE
    if _NC_CACHE is None:
        _NC_CACHE = build(B, K, V, N, M, TOPK, CORES)
    return _NC_CACHE


def kernel(query_key, W_k, b_k, assoc_matrix, usage_counter, memory_index,
           top_k=8, **_ignored):
    assert int(top_k) == TOPK
    nc = _get_nc()
    in_maps = make_inputs(query_key, W_k, b_k, assoc_matrix, usage_counter,
                          memory_index, B, K, V, N, M, TOPK, CORES)
    res = run_bass_kernel_spmd(nc, in_maps, list(range(CORES)))
    return assemble_outputs(res.results, B, V, TOPK, CORES)


# revision 4
# speedup vs baseline: 1.5719x; 1.5158x over previous
"""Trainium2 Bass kernel: EpisodicLearningSystem retrieval_knn (8-core SPMD).

Self-contained: kernel(**inputs) takes the full unsharded inputs, shards and
replicates across 8 NeuronCores, runs the Bass/Tile kernel, and returns
(retrieved_values, confidences, top_sims) matching the reference.
"""

import sys

for _p in ("/opt/trn_rl_repo", "/root/.axon_site/_ro/trn_rl_repo"):
    if _p not in sys.path:
        sys.path.insert(0, _p)



import numpy as np

import concourse.bacc as bacc
import concourse.bass as bass
import concourse.mybir as mybir
from concourse.ordered_set import OrderedSet
from concourse.tile import TileContext
from concourse.masks import make_identity

F32 = mybir.dt.float32
I32 = mybir.dt.int32
I16 = mybir.dt.int16
U32 = mybir.dt.uint32
SP = mybir.EngineType.SP
EPS = 1e-8


def build(B, K, V, N, M, TOPK, CORES, xsplit=2, upto='all'):
    assert K == 256 and V == 256 and TOPK == 8
    BL = B // CORES
    NL = N // CORES
    PAIRS = BL * TOPK
    assert BL % 16 == 0 and PAIRS % 128 == 0

    nc = bacc.Bacc(enable_partition_id=True, num_devices=CORES)

    qT = nc.dram_tensor("qT", [K, BL], F32, kind="ExternalInput")
    W0 = nc.dram_tensor("W0", [K, 128], F32, kind="ExternalInput")   # Wt cols k=2p
    W1 = nc.dram_tensor("W1", [K, 128], F32, kind="ExternalInput")   # Wt cols k=2p+1
    Wt = nc.dram_tensor("Wt", [K, K], F32, kind="ExternalInput")
    bk0 = nc.dram_tensor("bk0", [128, 1], F32, kind="ExternalInput")
    bk1 = nc.dram_tensor("bk1", [128, 1], F32, kind="ExternalInput")
    bkrow = nc.dram_tensor("bkrow", [1, K], F32, kind="ExternalInput")
    A = nc.dram_tensor("A", [N, K, V], F32, kind="ExternalInput")
    mi32 = nc.dram_tensor("mi32", [M, 1], I32, kind="ExternalInput")
    mi16w = nc.dram_tensor("mi16w", [BL, M // 16], I16, kind="ExternalInput")
    usage = nc.dram_tensor("usage", [N, 1], F32, kind="ExternalInput")

    rv_out = nc.dram_tensor("rv", [PAIRS, V], F32, kind="ExternalOutput")
    conf_out = nc.dram_tensor("conf", [BL, TOPK], F32, kind="ExternalOutput")
    tsim_out = nc.dram_tensor("tsim", [BL, TOPK], F32, kind="ExternalOutput")

    Aflat = A.ap().rearrange("n k v -> (n k v)")

    with TileContext(nc) as tc:
        with (
            tc.tile_pool(name="con", bufs=1) as con,
            tc.tile_pool(name="big", bufs=1) as bigp,
            tc.tile_pool(name="work", bufs=3) as work,
            tc.tile_pool(name="slotp", bufs=8) as slotp,
            tc.tile_pool(name="psum", bufs=2, space="PSUM") as psum,
            tc.tile_pool(name="psumr", bufs=1, space="PSUM") as psumr,
            tc.tile_pool(name="dram", bufs=1, space="DRAM") as dram,
        ):
            pid = nc.partition_id(
                engines=OrderedSet([SP, mybir.EngineType.Activation]))

            # ---- Phase X: bulk-read shard, DVE-extract keycol -> DRAM ----
            kc_loc = dram.tile([2, 128, NL], F32)
            ag_out = dram.tile([2 * CORES, 128, NL], F32)
            base = pid * (NL * K * V)
            kcT = [con.tile([128, NL], F32, tag=f"kcT{c}", name=f"kcT{c}")
                   for c in range(2)]
            XG = 8
            g0 = 0
            gi = 0
            while g0 < NL:
                G = min(XG, NL - g0)
                eng = nc.sync if gi % 2 == 0 else nc.scalar
                gt = work.tile([128, XG, 2, V], F32, tag="xgt", name="gt")
                eng.dma_start(
                    out=gt[:, :G],
                    in_=Aflat[bass.ds(base + g0 * (K * V), G * K * V)].rearrange(
                        "(g p c v) -> p g c v", g=G, p=128, c=2, v=V))
                for c in range(2):
                    nc.vector.tensor_copy(out=kcT[c][:, g0:g0 + G],
                                          in_=gt[:, :G, c, 0])
                g0 += G
                gi += 1
            for c in range(2):
                nc.sync.dma_start(out=kc_loc[c], in_=kcT[c][:])

            # ---- Phase AG ----
            nc.gpsimd.collective_compute(
                "AllGather", mybir.AluOpType.bypass,
                replica_groups=[list(range(CORES))],
                ins=[kc_loc.opt()], outs=[ag_out.opt()])

            # ---- load full knT ----
            knT = [bigp.tile([128, N], F32, tag=f"knT{c}", name=f"knT{c}") for c in range(2)]
            for cb in range(CORES):
                for c in range(2):
                    nc.sync.dma_start(out=knT[c][:, cb * NL:(cb + 1) * NL],
                                      in_=ag_out[2 * cb + c])

            # ---- constants ----
            ones_col = con.tile([128, 1], F32)
            nc.vector.memset(ones_col[:], 1.0)
            ones_row = con.tile([1, 128], F32)
            nc.vector.memset(ones_row[:], 1.0)
            ones_bl = con.tile([1, BL], F32)
            nc.vector.memset(ones_bl[:], 1.0)
            ident = con.tile([128, 128], F32)
            make_identity(nc, ident)

            # ---- Phase N: key norms -> scale knT in place ----
            BLKN = 500 if N % 500 == 0 else NL
            nblk = N // BLKN
            nrm_row = con.tile([1, N], F32)
            for blk in range(nblk):
                sl = slice(blk * BLKN, (blk + 1) * BLKN)
                pn = psum.tile([1, BLKN], F32, tag="psmall", name="pn")
                for c in range(2):
                    sq = work.tile([128, BLKN], F32, tag="sq")
                    nc.scalar.square(sq[:], knT[c][:, sl])
                    nc.tensor.matmul(out=pn[:], lhsT=ones_col[:], rhs=sq[:],
                                     start=(c == 0), stop=(c == 1))
                nc.vector.tensor_copy(out=nrm_row[:, sl], in_=pn[:])
            nc.scalar.sqrt(nrm_row[:], nrm_row[:])
            nc.vector.tensor_scalar_max(nrm_row[:], nrm_row[:], EPS)
            nc.vector.reciprocal(nrm_row[:], nrm_row[:])
            for blk in range(nblk):
                sl = slice(blk * BLKN, (blk + 1) * BLKN)
                pb = psum.tile([128, BLKN], F32, tag="pblk", name="pb")
                nc.tensor.matmul(out=pb[:], lhsT=ones_row[:], rhs=nrm_row[:, sl],
                                 start=True, stop=True)
                for c in range(2):
                    nc.vector.tensor_mul(out=knT[c][:, sl], in0=knT[c][:, sl], in1=pb[:])

            # ---- Phase E: encoder ----
            qTt = con.tile([128, 2, BL], F32)
            nc.sync.dma_start(out=qTt[:], in_=qT.ap().rearrange("(c p) b -> p c b", p=128))
            w0t = con.tile([128, 2, 128], F32)
            nc.sync.dma_start(out=w0t[:], in_=W0.ap().rearrange("(c p) k -> p c k", p=128))
            w1t = con.tile([128, 2, 128], F32)
            nc.sync.dma_start(out=w1t[:], in_=W1.ap().rearrange("(c p) k -> p c k", p=128))
            wtt = con.tile([128, 2, K], F32)
            nc.sync.dma_start(out=wtt[:], in_=Wt.ap().rearrange("(c p) k -> p c k", p=128))
            bk0t = con.tile([128, 1], F32)
            nc.sync.dma_start(out=bk0t[:], in_=bk0[:])
            bk1t = con.tile([128, 1], F32)
            nc.sync.dma_start(out=bk1t[:], in_=bk1[:])
            bkrt = con.tile([1, K], F32)
            nc.sync.dma_start(out=bkrt[:], in_=bkrow[:])

            eT = [con.tile([128, BL], F32, tag=f"eT{c}", name=f"eT{c}") for c in range(2)]
            for c, wt_ in ((0, w0t), (1, w1t)):
                pe = psum.tile([128, BL], F32, tag="psmall", name="pe")
                for jc in range(2):
                    nc.tensor.matmul(out=pe[:], lhsT=wt_[:, jc, :], rhs=qTt[:, jc, :],
                                     start=(jc == 0), stop=(jc == 1))
                nc.scalar.activation(eT[c][:], pe[:], mybir.ActivationFunctionType.Tanh,
                                     bias=(bk0t[:, 0:1] if c == 0 else bk1t[:, 0:1]))

            # e in [b, k] layout for query norms
            peb = psum.tile([BL, K], F32, tag="psmall", name="peb")
            for jc in range(2):
                nc.tensor.matmul(out=peb[:], lhsT=qTt[:, jc, :], rhs=wtt[:, jc, :],
                                 start=(jc == 0), stop=False)
            nc.tensor.matmul(out=peb[:], lhsT=ones_bl[:], rhs=bkrt[:],
                             start=False, stop=True)
            e_b = con.tile([BL, K], F32)
            nc.scalar.activation(e_b[:], peb[:], mybir.ActivationFunctionType.Tanh)
            esq = work.tile([BL, K], F32, tag="esq")
            nc.scalar.square(esq[:], e_b[:])
            rq = con.tile([BL, 1], F32)
            nc.vector.reduce_sum(out=rq[:], in_=esq[:], axis=mybir.AxisListType.X)
            nc.scalar.sqrt(rq[:], rq[:])
            nc.vector.tensor_scalar_max(rq[:], rq[:], EPS)
            nc.vector.reciprocal(rq[:], rq[:])

            # ---- Phase S: slot sims ----
            ssims = bigp.tile([BL, N], F32)
            SBLK = 500 if N % 500 == 0 else NL
            for blk in range(N // SBLK):
                sl = slice(blk * SBLK, (blk + 1) * SBLK)
                ps = psum.tile([BL, SBLK], F32, tag="pblk", name="ps")
                for c in range(2):
                    nc.tensor.matmul(out=ps[:], lhsT=eT[c][:], rhs=knT[c][:, sl],
                                     start=(c == 0), stop=(c == 1))
                nc.vector.tensor_scalar_mul(ssims[:, sl], ps[:], rq[:, 0:1])

            # ---- Phase G: expand to m-space ----
            mi16t = con.tile([BL, M // 16], I16)
            nc.sync.dma_start(out=mi16t[:], in_=mi16w[:])
            msims = bigp.tile([BL, M], F32)
            nc.gpsimd.ap_gather(
                msims[:].rearrange("p (m one) -> p m one", one=1),
                ssims[:].rearrange("p (n one) -> p n one", one=1),
                mi16t[:], channels=BL, num_elems=N, d=1, num_idxs=M)

            # ---- Phase T: top-8 ----
            tsim = con.tile([BL, 8], F32)
            msel = con.tile([BL, 8], U32)
            nc.vector.max(out=tsim[:], in_=msims[:])
            nc.vector.max_index(out=msel[:], in_max=tsim[:], in_values=msims[:])
            nc.sync.dma_start(out=tsim_out[:], in_=tsim[:])

            # ---- Phase C: actual slots, usage, confidences ----
            act_t = con.tile([BL, 8, 1], I32)
            for kk in range(8):
                nc.gpsimd.indirect_dma_start(
                    out=act_t[:, kk], out_offset=None, in_=mi32[:],
                    in_offset=bass.IndirectOffsetOnAxis(ap=msel[:, kk:kk + 1], axis=0))
            usel = con.tile([BL, 8, 1], F32)
            for kk in range(8):
                nc.gpsimd.indirect_dma_start(
                    out=usel[:, kk], out_offset=None, in_=usage[:],
                    in_offset=bass.IndirectOffsetOnAxis(ap=act_t[:, kk, 0:1], axis=0))
            lus = con.tile([BL, 8], F32)
            nc.scalar.activation(lus[:], usel[:, :, 0],
                                 mybir.ActivationFunctionType.Ln, bias=1.0)
            nc.vector.tensor_scalar_add(lus[:], lus[:], 1.0)
            conf = con.tile([BL, 8], F32)
            nc.vector.tensor_mul(out=conf[:], in0=tsim[:], in1=lus[:])
            nc.sync.dma_start(out=conf_out[:], in_=conf[:])

            # ---- Phase R: retrieval ----
            ACT_E = mybir.EngineType.Activation
            off_t = con.tile([BL, 8, 1], I32)
            if upto == 'all':
                nc.vector.tensor_scalar_mul(off_t[:, :, 0], act_t[:, :, 0], K * V)
            pr = [psumr.tile([128, PAIRS], F32, tag=f"pr{vh}", name=f"pr{vh}")
                  for vh in range(2)] if upto == 'all' else None
            for b in (range(BL) if upto == 'all' else []):
                half = 0 if b < BL // 2 else 1
                eng = nc.sync if half == 0 else nc.scalar
                eng_t = SP if half == 0 else ACT_E
                for kk in range(TOPK):
                    j = b * TOPK + kk
                    with eng.register(f"roff{j}") as r:
                        eng.reg_load(r, off_t[b:b + 1, kk, 0:1])
                        sv = nc.snap(r, engines=OrderedSet([eng_t]),
                                     min_val=0, max_val=(N - 1) * K * V)
                    st = slotp.tile([128, 2, V], F32, tag=f"slot{half}",
                                    name=f"st{half}")
                    eng.dma_start(
                        out=st[:],
                        in_=Aflat[bass.ds(sv, K * V)].rearrange(
                            "(p c v) -> p c v", p=128, c=2, v=V))
                    for vh in range(2):
                        for c in range(2):
                            nc.tensor.matmul(
                                out=pr[vh][:, j:j + 1],
                                lhsT=st[:, c, vh * 128:(vh + 1) * 128],
                                rhs=eT[c][:, b:b + 1],
                                start=(c == 0), stop=(c == 1))

            # ---- Phase O: transpose + write retrieved values ----
            rT = []
            for vh in (range(2) if upto == 'all' else []):
                t = bigp.tile([128, PAIRS], F32, tag=f"rT{vh}", name=f"rT{vh}")
                nc.vector.tensor_copy(out=t[:], in_=pr[vh][:])
                rT.append(t)
            for pc in (range(PAIRS // 128) if upto == 'all' else []):
                rvs = work.tile([128, V], F32, tag="rvs")
                for vh in range(2):
                    pt = psum.tile([128, 128], F32, tag="psmall", name="pt")
                    nc.tensor.transpose(out=pt[:], in_=rT[vh][:, pc * 128:(pc + 1) * 128],
                                        identity=ident[:])
                    nc.vector.tensor_copy(out=rvs[:, vh * 128:(vh + 1) * 128], in_=pt[:])
                nc.sync.dma_start(out=rv_out[pc * 128:(pc + 1) * 128, :], in_=rvs[:])

    nc.finalize()
    return nc


def make_inputs(q, W, b, A, usage, mi, B, K, V, N, M, TOPK, CORES):
    """Build per-core input maps (host-side sharding/layout only)."""
    BL = B // CORES
    q = np.asarray(q, dtype=np.float32)
    W = np.asarray(W, dtype=np.float32)
    b = np.asarray(b, dtype=np.float32)
    A = np.ascontiguousarray(np.asarray(A, dtype=np.float32))
    usage = np.asarray(usage, dtype=np.float32)
    mi = np.asarray(mi).astype(np.int32)

    Wt = np.ascontiguousarray(W.T)
    W0 = np.ascontiguousarray(Wt[:, 0::2])
    W1 = np.ascontiguousarray(Wt[:, 1::2])
    bk0 = np.ascontiguousarray(b[0::2][:, None])
    bk1 = np.ascontiguousarray(b[1::2][:, None])
    bkrow = np.ascontiguousarray(b[None, :])
    mi32 = np.ascontiguousarray(mi[:, None])
    w16 = np.ascontiguousarray(mi.astype(np.int16).reshape(M // 16, 16).T)  # [16, M//16]
    mi16w = np.ascontiguousarray(np.tile(w16, (BL // 16, 1)))
    usage2 = np.ascontiguousarray(usage[:, None])

    in_maps = []
    for c in range(CORES):
        qTl = np.ascontiguousarray(q[c * BL:(c + 1) * BL].T)
        in_maps.append({
            "qT": qTl, "W0": W0, "W1": W1, "Wt": Wt,
            "bk0": bk0, "bk1": bk1, "bkrow": bkrow,
            "A": A, "mi32": mi32, "mi16w": mi16w, "usage": usage2,
        })
    return in_maps


def assemble_outputs(results, B, V, TOPK, CORES):
    BL = B // CORES
    rv = np.concatenate([r["rv"].reshape(BL, TOPK, V) for r in results], axis=0)
    conf = np.concatenate([r["conf"] for r in results], axis=0)
    tsim = np.concatenate([r["tsim"] for r in results], axis=0)
    return rv, conf, tsim


from concourse.bass_utils import run_bass_kernel_spmd

B, K, V, N, M, TOPK, CORES = 256, 256, 256, 4000, 4000, 8, 8

_NC_CACHE = None


def _get_nc():
    global _NC_CACHE
    if _NC_CACHE is None:
        _NC_CACHE = build(B, K, V, N, M, TOPK, CORES)
    return _NC_CACHE


def kernel(query_key, W_k, b_k, assoc_matrix, usage_counter, memory_index,
           top_k=8, **_ignored):
    assert int(top_k) == TOPK
    nc = _get_nc()
    in_maps = make_inputs(query_key, W_k, b_k, assoc_matrix, usage_counter,
                          memory_index, B, K, V, N, M, TOPK, CORES)
    res = run_bass_kernel_spmd(nc, in_maps, list(range(CORES)))
    return assemble_outputs(res.results, B, V, TOPK, CORES)
